# revision 1
# baseline (speedup 1.0000x reference)
"""Attention-LSTM decoder (LAS-style) Trainium2 Bass kernel.

Sharding: data-parallel over batch N=64 -> 8 cores x 8 examples.

Device strategy (per core, b=8 examples):
 - Recurrent matmuls are batch-stationary (lhsT = state columns [K, 8]) with
   weights streamed as float32r (full-rate fp32 for N>=256).
 - emb(x_t) @ W_ih1[:, :512].T is precomputed for all steps in phase A
   (indirect-DMA gather + big matmul), biases folded in.
 - sigmoid via tanh identity; states stored scaled by 2 (S=2c, H=2h) so each
   gate is one scalar_tensor_tensor; the 2x on h is compensated by
   pre-halving W_hh1/W_ih2/W_hh2/keys/W_out[:, :128] on the host.
 - Attention energy computed DENSE [8, 500] by accumulating 8 matmuls whose
   stationary is H2t masked to one column (block-diag trick); pad mask added
   via an I8 matmul; exp uses accum_out for the softmax denominator (no max
   subtraction -- energies are small, verified against the reference).
 - ctx computed per (example, T-chunk) with stationary val chunks; lands
   transposed [128, 8] = exactly the layout the next-step z1 matmul and the
   final output matmul need.
 - Phase C: [1600, 256] @ [256, 8000] from the stored h2/ctx histories.
"""

import os
from contextlib import ExitStack

import numpy as np

V, H, KS, VS, T, N, L = 8000, 512, 128, 128, 500, 64, 200
NCORES = 8
B = N // NCORES          # 8 examples per core
TCH = 4                  # T chunks for ctx matmuls
TSUB = T // TCH          # 125
HCH = H // 128           # 4 chunks of the h1 dim
G1 = 4 * H               # 2048
G2 = 4 * KS              # 512
NEG = -1e9

_cache = {}


def _nt_mch(nsteps):
    nt = B * nsteps
    return nt, (nt + 127) // 128


def _prep_inputs(nsteps, key, values, lens, text, emb, W_ih1, W_hh1, b_ih1,
                 b_hh1, W_ih2, W_hh2, b_ih2, b_hh2, W_out, b_out):
    """Host-side layout prep. Returns per-core list of input dicts."""
    f = np.float32
    nt, mch = _nt_mch(nsteps)
    W_ih1 = np.asarray(W_ih1, f)
    W_hh1 = np.asarray(W_hh1, f)
    W_ih2 = np.asarray(W_ih2, f)
    W_hh2 = np.asarray(W_hh2, f)
    W_out = np.asarray(W_out, f)
    emb = np.ascontiguousarray(np.asarray(emb, f))

    # z1 moving chunks: [128, 5, 2048]; k-chunk 0 = W_ctx.T, 1..4 = W_hh1.T/2
    W1r = np.empty((128, 5, G1), f)
    W1r[:, 0, :] = W_ih1[:, H:H + VS].T
    for j in range(4):
        W1r[:, 1 + j, :] = 0.5 * W_hh1[:, 128 * j:128 * (j + 1)].T
    # z2 moving chunks: [128, 5, 512]; 0..3 = W_ih2.T/2, 4 = W_hh2.T/2
    W2r = np.empty((128, 5, G2), f)
    for j in range(4):
        W2r[:, j, :] = 0.5 * W_ih2[:, 128 * j:128 * (j + 1)].T
    W2r[:, 4, :] = 0.5 * W_hh2.T
    WembT = np.ascontiguousarray(W_ih1[:, :H].T.reshape(4, 128, G1)
                                 .transpose(1, 0, 2))        # [128, 4, 2048]
    WoutT = np.empty((128, 2, V), f)
    WoutT[:, 0, :] = 0.5 * W_out[:, :KS].T
    WoutT[:, 1, :] = W_out[:, KS:].T

    b1row = (np.asarray(b_ih1, f) + np.asarray(b_hh1, f)).reshape(1, G1)
    b2row = (np.asarray(b_ih2, f) + np.asarray(b_hh2, f)).reshape(1, G2)
    boutrow = np.ascontiguousarray(np.asarray(b_out, f).reshape(1, V))

    ident = np.eye(128, dtype=f)
    onesr = np.ones((1, 128), f)
    dmask = np.zeros((128, B * B), f)
    for n in range(B):
        dmask[:, B * n + n] = 1.0

    shared = dict(emb=emb, W1r=W1r, W2r=W2r, WembT=WembT, WoutT=WoutT,
                  b1row=b1row, b2row=b2row, boutrow=boutrow,
                  ident=ident, onesr=onesr, dmask=dmask)

    per_core = []
    for c in range(NCORES):
        sl = slice(B * c, B * (c + 1))
        k_c = np.asarray(key[:, sl, :], f)       # (500, 8, 128)
        v_c = np.asarray(values[:, sl, :], f)
        lens_c = np.asarray(lens[sl])
        text_c = np.asarray(text[sl, :])
        keysT = np.ascontiguousarray(0.5 * k_c.transpose(2, 1, 0))
        vals = np.ascontiguousarray(
            v_c.reshape(TCH, TSUB, B, VS).transpose(1, 2, 0, 3))
        mask8 = np.where(np.arange(T)[None, :] >= lens_c[:, None], NEG, 0.0)
        mask8 = np.ascontiguousarray(mask8.astype(f))
        # gather indices, t-major: idx[t*8+n] = text[n, t]; SBUF [128, mch]
        tidx = text_c.T[:nsteps].reshape(nt).astype(np.int32)
        tidx_pad = np.zeros(128 * mch, np.int32)
        tidx_pad[:nt] = tidx
        tidx_sb = np.ascontiguousarray(tidx_pad.reshape(mch, 128).T)
        ctx0T = np.ascontiguousarray(v_c[0].T)
        d = dict(shared)
        d.update(keysT=keysT, vals=vals, mask8=mask8, tidx=tidx_sb,
                 ctx0T=ctx0T)
        per_core.append(d)
    return per_core


def build(ctx: ExitStack, tc, out_ap, ins, nsteps=L):
    import concourse.bass as bass
    from concourse import mybir

    ablate = set(os.environ.get("DEC_ABLATE", "").split(","))

    nc = tc.nc
    f32 = mybir.dt.float32
    f32r = mybir.dt.float32r
    AF = mybir.ActivationFunctionType
    OP = mybir.AluOpType
    nt, mch = _nt_mch(nsteps)

    mm = nc.tensor.matmul

    consts = ctx.enter_context(tc.tile_pool(name="consts", bufs=1))
    hists = ctx.enter_context(tc.tile_pool(name="hists", bufs=1))
    dram = ctx.enter_context(tc.tile_pool(name="dram", bufs=1, space="DRAM"))

    def load_const(name, dtype=f32):
        a = ins[name]
        t = consts.tile(list(a.shape), dtype, tag=name)
        nc.sync.dma_start(t[:], a[:])
        return t

    W1r = load_const("W1r", f32r)        # [128, 5, 2048]
    W2r = load_const("W2r", f32r)        # [128, 5, 512]
    keysT = load_const("keysT", f32r)    # [128, 8, 500]
    vals = load_const("vals")      # [125, 8, 4, 128] fp32 (N=1 MMs)
    mask8 = load_const("mask8", f32r)    # [8, 500]
    b2row = load_const("b2row", f32r)    # [1, 512]
    ident = load_const("ident")    # [128, 128] fp32, for transposes
    onesr = load_const("onesr", f32r)    # [1, 128]
    dmask = load_const("dmask", f32r)    # [128, 64]
    ctx0T = load_const("ctx0T", f32r)    # [128, 8]
    tidx = load_const("tidx", mybir.dt.int32)   # [128, mch]
    identr = consts.tile([128, 128], f32r, tag="identr")
    nc.gpsimd.dma_start(identr[:], ins["ident"][:])

    # histories: slot s holds the state after step s-1 (slot 0 = initial)
    H2h = hists.tile([128, B * (nsteps + 1)], f32r)
    CXh = hists.tile([128, B * (nsteps + 1)], f32r)
    H1t = hists.tile([128, 2, HCH * B], f32r)   # ping-pong h1T (2h scale)
    S1 = hists.tile([B, 2, H], f32)             # 2*c1
    S2 = hists.tile([B, 2, KS], f32)            # 2*c2
    nc.gpsimd.memset(H2h[:, 0:B].bitcast(f32), 0.0)
    nc.vector.tensor_copy(CXh[:, 0:B], ctx0T[:])
    nc.gpsimd.memset(H1t[:, 0, :].bitcast(f32), 0.0)
    nc.gpsimd.memset(S1[:, 0, :], 0.0)
    nc.gpsimd.memset(S2[:, 0, :], 0.0)

    zemb_d = dram.tile([128 * mch, G1], f32r)
    I8 = ident[0:B, 0:B]
    I8r = identr[0:B, 0:B]

    # ============ phase A: z_emb = emb[text] @ WembT + b1 ================
    with tc.tile_pool(name="pha", bufs=1) as pha, \
         tc.tile_pool(name="pha_g", bufs=3) as pha_g, \
         tc.tile_pool(name="pha_t", bufs=6) as pha_t, \
         tc.tile_pool(name="pha_z", bufs=2) as pha_z, \
         tc.tile_pool(name="pha_ps", bufs=3, space="PSUM") as pha_ps, \
         tc.tile_pool(name="pha_ps2", bufs=4, space="PSUM") as pha_ps2:
        WembT = pha.tile([128, 4, G1], f32r)
        nc.sync.dma_start(WembT[:], ins["WembT"][:])
        b1row = pha.tile([1, G1], f32r)
        nc.sync.dma_start(b1row[:], ins["b1row"][:])
        emb_ap = ins["emb"]

        for m in range(mch):
            gath = pha_g.tile([128, H], f32, tag="g")
            nc.gpsimd.indirect_dma_start(
                out=gath[:], out_offset=None, in_=emb_ap[:],
                in_offset=bass.IndirectOffsetOnAxis(
                    ap=tidx[:, m:m + 1], axis=0))
            zt = pha_z.tile([128, G1], f32, tag="z")
            embT = []
            for kc in range(4):
                tp = pha_ps.tile([128, 128], f32, tag="tp")
                nc.tensor.transpose(tp[:], gath[:, 128 * kc:128 * (kc + 1)],
                                    ident[:])
                et = pha_t.tile([128, 128], f32r, tag="et")
                nc.vector.tensor_copy(et[:], tp[:])
                embT.append(et)
            for q in range(4):
                ps = pha_ps2.tile([128, 512], f32, tag="zps")
                sl = slice(512 * q, 512 * (q + 1))
                mm(ps[:], onesr[:, 0:128], b1row[:, sl],
                   start=True, stop=False)
                for kc in range(4):
                    mm(ps[:], embT[kc][:], WembT[:, kc, sl],
                       start=False, stop=(kc == 3))
                nc.scalar.copy(zt[:, sl], ps[:])
            nc.gpsimd.dma_start(zemb_d[128 * m:128 * (m + 1), :], zt[:])

    # ============ phase B: the recurrence ================================
    with tc.tile_pool(name="zemb", bufs=3) as zemb_p, \
         tc.tile_pool(name="gates", bufs=3) as gates, \
         tc.tile_pool(name="small", bufs=3) as small, \
         tc.tile_pool(name="ps_z1", bufs=1, space="PSUM") as ps_z1, \
         tc.tile_pool(name="ps_z2", bufs=1, space="PSUM") as ps_z2, \
         tc.tile_pool(name="ps_en", bufs=1, space="PSUM") as ps_en, \
         tc.tile_pool(name="ps_sm", bufs=1, space="PSUM") as ps_sm:

        z1_ps = ps_z1.tile([B, G1], f32)
        z2_ps = ps_z2.tile([B, G2], f32)
        en_ps = ps_en.tile([B, T], f32)

        def lstm_gates(z_ps, S, pp, w, h_out):
            """z_ps [B, 4w] PSUM -> h_out [B, w] (= 2h). Gate order i,f,g,o."""
            ti = gates.tile([B, w], f32, tag=f"ti{w}")
            tf = gates.tile([B, w], f32, tag=f"tf{w}")
            tg = gates.tile([B, w], f32, tag=f"tg{w}")
            to = gates.tile([B, w], f32, tag=f"to{w}")
            nc.scalar.activation(tf[:], z_ps[:, w:2 * w], AF.Tanh, scale=0.5)
            nc.scalar.activation(tg[:], z_ps[:, 2 * w:3 * w], AF.Tanh)
            nc.scalar.activation(ti[:], z_ps[:, 0:w], AF.Tanh, scale=0.5)
            nc.scalar.activation(to[:], z_ps[:, 3 * w:4 * w], AF.Tanh,
                                 scale=0.5)
            fc = gates.tile([B, w], f32, tag=f"fc{w}")
            u = gates.tile([B, w], f32, tag=f"u{w}")
            tcn = gates.tile([B, w], f32, tag=f"tc{w}")
            nc.vector.scalar_tensor_tensor(fc[:], tf[:], 1.0, S[:, pp ^ 1, :],
                                           op0=OP.add, op1=OP.mult)
            nc.vector.scalar_tensor_tensor(u[:], ti[:], 1.0, tg[:],
                                           op0=OP.add, op1=OP.mult)
            nc.vector.scalar_tensor_tensor(S[:, pp, :], fc[:], 0.5, u[:],
                                           op0=OP.mult, op1=OP.add)
            nc.scalar.activation(tcn[:], S[:, pp, :], AF.Tanh, scale=0.5)
            nc.vector.scalar_tensor_tensor(h_out[:], to[:], 1.0, tcn[:],
                                           op0=OP.add, op1=OP.mult)

        for t in range(nsteps):
            pp = (t + 1) % 2
            po = t % 2
            # -- z1 ----------------------------------------------------
            if "nodma" not in ablate:
                zt = zemb_p.tile([B, G1], f32r, tag="zemb")
                nc.sync.dma_start(zt[:], zemb_d[B * t:B * (t + 1), :])
            else:
                if t == 0:
                    zt_c = hists.tile([B, G1], f32r)
                    nc.gpsimd.memset(zt_c[:].bitcast(f32), 0.01)
                zt = zt_c
            cxT = CXh[:, B * t:B * (t + 1)]
            for q in range(4):
                sl = slice(512 * q, 512 * (q + 1))
                mm(z1_ps[:, sl], I8r, zt[:, sl], start=True, stop=False)
                mm(z1_ps[:, sl], cxT, W1r[:, 0, sl],
                   start=False, stop=False)
                for j in range(4):
                    mm(z1_ps[:, sl], H1t[:, po, B * j:B * (j + 1)],
                       W1r[:, 1 + j, sl], start=False, stop=(j == 3))
            # -- gates 1; h1 rows -> H1t --------------------------------
            h1r = gates.tile([B, H], f32, tag="h1r")
            lstm_gates(z1_ps, S1, pp, H, h1r)
            tp = ps_sm.tile([128, 4 * B], f32, tag="tp")
            for j in range(HCH):
                nc.tensor.transpose(tp[:, B * j:B * (j + 1)],
                                    h1r[:, 128 * j:128 * (j + 1)], I8)
            nc.vector.tensor_copy(H1t[:, pp, :], tp[:])
            # -- z2 ----------------------------------------------------
            mm(z2_ps[:], onesr[:, 0:B], b2row[:], start=True,
               stop=False)
            for j in range(4):
                mm(z2_ps[:], H1t[:, pp, B * j:B * (j + 1)],
                   W2r[:, j, :], start=False, stop=False)
            mm(z2_ps[:], H2h[:, B * t:B * (t + 1)], W2r[:, 4, :],
               start=False, stop=True)
            # -- gates 2; h2 rows -> H2h slot t+1 ----------------------
            h2r = gates.tile([B, KS], f32, tag="h2r")
            lstm_gates(z2_ps, S2, pp, KS, h2r)
            h2T = H2h[:, B * (t + 1):B * (t + 2)]
            tp2 = ps_sm.tile([128, 4 * B], f32, tag="tp")
            nc.tensor.transpose(tp2[:, 0:B], h2r[:], I8)
            nc.vector.tensor_copy(h2T, tp2[:, 0:B])
            if "noattn" in ablate:
                nc.vector.tensor_copy(CXh[:, B * (t + 1):B * (t + 2)],
                                      CXh[:, B * t:B * (t + 1)])
                continue
            # -- energy: dense [8, 500] --------------------------------
            zh2 = small.tile([128, B, B], f32r, tag="zh2")
            nc.vector.tensor_tensor(
                zh2[:],
                h2T.rearrange("p (a n) -> p a n", a=1).to_broadcast(
                    [128, B, B]),
                dmask[:].rearrange("p (a b) -> p a b", a=B), op=OP.mult)
            mm(en_ps[:], I8r, mask8[:], start=True, stop=False)
            for n in range(B):
                mm(en_ps[:], zh2[:, n, :], keysT[:, n, :],
                   start=False, stop=(n == B - 1))
            # -- softmax -----------------------------------------------
            att = small.tile([B, T], f32, tag="att")
            atts = small.tile([B, T], f32, tag="atts")
            den = small.tile([B, 1], f32, tag="den")
            rden = small.tile([B, 1], f32, tag="rden")
            nc.scalar.activation(att[:], en_ps[:], AF.Exp, accum_out=den[:])
            nc.vector.reciprocal(rden[:], den[:])
            nc.vector.tensor_scalar_mul(atts[:], att[:], rden[:, 0:1])
            # -- attn transpose + ctx ----------------------------------
            tp3 = ps_sm.tile([128, 4 * B], f32, tag="tp")
            for cch in range(TCH):
                nc.tensor.transpose(tp3[0:TSUB, B * cch:B * (cch + 1)],
                                    atts[:, TSUB * cch:TSUB * (cch + 1)], I8)
            attT = small.tile([128, TCH, B], f32, tag="attT")
            nc.vector.tensor_copy(
                attT[0:TSUB].rearrange("p a b -> p (a b)"), tp3[0:TSUB, :])
            cx_ps = ps_sm.tile([128, B], f32, tag="cxps")
            for n in range(B):
                for cch in range(TCH):
                    mm(cx_ps[:, n:n + 1], vals[:, n, cch, :],
                       attT[0:TSUB, cch, n:n + 1],
                       start=(n == 0 and cch == 0),
                       stop=(n == B - 1 and cch == TCH - 1))
            nc.vector.tensor_copy(CXh[:, B * (t + 1):B * (t + 2)], cx_ps[:])

    # ============ phase C: logits ========================================
    with tc.tile_pool(name="phc_w", bufs=2) as phc_w, \
         tc.tile_pool(name="phc_o", bufs=3) as phc_o, \
         tc.tile_pool(name="phc_ps", bufs=8, space="PSUM") as phc_ps:
        vchunks = [(512 * q, min(512, V - 512 * q))
                   for q in range((V + 511) // 512)]
        groups = [vchunks[i:i + 4] for i in range(0, len(vchunks), 4)]
        for grp in groups:
            g0 = grp[0][0]
            gw = sum(w for _, w in grp)
            wg = phc_w.tile([128, 2, 2048], f32r, tag="wg")
            nc.sync.dma_start(wg[:, :, 0:gw], ins["WoutT"][:, :, g0:g0 + gw])
            bg = phc_w.tile([1, 2048], f32r, tag="bg")
            nc.sync.dma_start(bg[:, 0:gw], ins["boutrow"][:, g0:g0 + gw])
            for m in range(mch):
                rows = min(128, nt - 128 * m)
                h2blk = H2h[:, B + 128 * m:B + 128 * m + rows]
                cxblk = CXh[:, B + 128 * m:B + 128 * m + rows]
                ot = phc_o.tile([128, 2048], f32, tag="ot")
                for qi, (q0, qw) in enumerate(grp):
                    nsl = slice(q0 - g0, q0 - g0 + qw)
                    ps = phc_ps.tile([128, 512], f32, tag="lg")
                    mm(ps[0:rows, 0:qw], onesr[:, 0:rows],
                       bg[:, nsl], start=True, stop=False)
                    mm(ps[0:rows, 0:qw], h2blk, wg[:, 0, nsl],
                       start=False, stop=False)
                    mm(ps[0:rows, 0:qw], cxblk, wg[:, 1, nsl],
                       start=False, stop=True)
                    if qi % 2 == 0:
                        nc.scalar.copy(ot[0:rows, nsl], ps[0:rows, 0:qw])
                    else:
                        nc.vector.tensor_copy(ot[0:rows, nsl],
                                              ps[0:rows, 0:qw])
                nc.sync.dma_start(out_ap[128 * m:128 * m + rows, g0:g0 + gw],
                                  ot[0:rows, 0:gw])


def _build_program(nsteps):
    import concourse.tile as tile
    from concourse import bacc, mybir

    nt, mch = _nt_mch(nsteps)
    nc = bacc.Bacc("TRN2", target_bir_lowering=False, debug=False,
                   num_devices=NCORES)
    shapes = dict(
        emb=(V, H), W1r=(128, 5, G1), W2r=(128, 5, G2), WembT=(128, 4, G1),
        WoutT=(128, 2, V), b1row=(1, G1), b2row=(1, G2), boutrow=(1, V),
        ident=(128, 128), onesr=(1, 128), dmask=(128, B * B),
        keysT=(128, B, T), vals=(TSUB, B, TCH, VS), mask8=(B, T),
        ctx0T=(128, B),
    )
    F32R_INS = {"W1r", "W2r", "WembT", "WoutT", "b1row", "b2row", "boutrow",
                "onesr", "dmask", "keysT", "mask8", "ctx0T"}
    ins = {}
    for name, shp in shapes.items():
        dt_ = mybir.dt.float32r if name in F32R_INS else mybir.dt.float32
        ins[name] = nc.dram_tensor(name, list(shp), dt_,
                                   kind="ExternalInput").ap()
    ins["tidx"] = nc.dram_tensor("tidx", [128, mch], mybir.dt.int32,
                                 kind="ExternalInput").ap()
    out = nc.dram_tensor("out", [nt, V], mybir.dt.float32,
                         kind="ExternalOutput").ap()
    with ExitStack() as ctx:
        tc = ctx.enter_context(tile.TileContext(nc))
        build(ctx, tc, out, ins, nsteps=nsteps)
    nc.compile()
    return nc


def kernel(**inputs) -> np.ndarray:
    from concourse.bass_utils import run_bass_kernel_spmd

    nsteps = int(os.environ.get("DEC_NSTEPS", L))
    per_core = _prep_inputs(nsteps, **inputs)
    if nsteps not in _cache:
        _cache[nsteps] = _build_program(nsteps)
    nc = _cache[nsteps]
    res = run_bass_kernel_spmd(
        nc, per_core, core_ids=list(range(NCORES)),
        trace=bool(int(os.environ.get("DEC_TRACE", "0"))),
    )
    outs = []
    for c in range(NCORES):
        o = res.results[c]["out"]        # [nt, 8000], rows t*8+n
        outs.append(o.reshape(nsteps, B, V).transpose(1, 0, 2))
    full = np.concatenate(outs, axis=0)  # (64, nsteps, 8000)
    kernel.last_results = res
    return full



# revision 25
# speedup vs baseline: 2.3906x; 2.3906x over previous
"""Attention-LSTM decoder (LAS-style) Trainium2 Bass kernel, v3.

Sharding: data-parallel over batch N=64 -> 8 cores x 8 examples.

v3 design (vs the v1 baseline):
 - z1 matmuls are column-tiled 4-way (tile_position col groups 0/32/64/96)
   with the 2048 gate columns reordered so group g holds [i|f|g|o] for
   h-dims [128g, 128g+128).  The four groups stream their weight chunks
   concurrently (separate XBUSes), and the LSTM1 gate math then runs on a
   [104, *] partition span (32 active lanes instead of 8) in 6 fused ops.
 - All 4 tanh's per cell fused into one activation: the g-gate columns of
   the weights are pre-doubled on the host so a single scale=0.5 works.
 - No pad mask matmul: keys/vals are zeroed at padded positions on the
   host, so padded energies are exactly 0 -> exp = 1, and the softmax
   denominator is corrected by subtracting npad = T - len per example.
 - The 1/den normalization folds into the attention transpose by using
   diag(rden) as the moving operand instead of the identity.
 - att/attT/vals in bf16: the ctx matmuls then get fast-weight-load
   (128-col bf16 stationaries), cutting their LDWEIGHTS cost ~4x.
 - PSUM bank has_written discipline: each shared accumulation bank is
   opened with one zeroing matmul (ones-column x zero-row), then every
   real matmul accumulates with start=False.
 - z1 for step t+1 (zero + zemb-inject + h1 chunks) is emitted after
   energy so it executes under the softmax/attention window; only the 4
   ctx-chunk matmuls sit on the recurrence critical path.
"""

import os
from contextlib import ExitStack

import numpy as np

V, H, KS, VS, T, N, L = 8000, 512, 128, 128, 500, 64, 200
NCORES = 8
B = N // NCORES          # 8 examples per core
TP = 512                 # padded T (4 chunks of 128)
TCH = 4
G1 = 4 * H               # 2048
G2 = 4 * KS              # 512

_cache = {}


def _nt_mch(nsteps):
    nt = B * nsteps
    return nt, (nt + 127) // 128


def _z1_perm():
    """new column -> old column map for the reordered z1 gate layout."""
    p = np.empty(G1, np.int64)
    for g in range(4):          # h-block
        for gate in range(4):   # i, f, g, o
            for j in range(128):
                h = 128 * g + j
                p[512 * g + 128 * gate + j] = 512 * gate + h
    return p


def _prep_inputs(nsteps, key, values, lens, text, emb, W_ih1, W_hh1, b_ih1,
                 b_hh1, W_ih2, W_hh2, b_ih2, b_hh2, W_out, b_out):
    """Host-side layout prep. Returns per-core list of input dicts."""
    f = np.float32
    nt, mch = _nt_mch(nsteps)
    W_ih1 = np.asarray(W_ih1, f)
    W_hh1 = np.asarray(W_hh1, f)
    W_ih2 = np.asarray(W_ih2, f)
    W_hh2 = np.asarray(W_hh2, f)
    W_out = np.asarray(W_out, f)
    emb = np.ascontiguousarray(np.asarray(emb, f))
    lens = np.asarray(lens)

    perm = _z1_perm()
    # doubling vector for the g-gate columns in the NEW layout
    dbl = np.ones(G1, f)
    for g in range(4):
        dbl[512 * g + 256:512 * g + 384] = 2.0
    dbl2 = np.ones(G2, f)
    dbl2[256:384] = 2.0     # g-gate of LSTM2 (layout unchanged)

    # z1 moving chunks: [128, 5, 2048]; k-chunk 0 = W_ctx.T, 1..4 = W_hh1.T/2
    W1r = np.empty((128, 5, G1), f)
    W1r[:, 0, :] = (W_ih1[:, H:H + VS].T)[:, perm] * dbl
    for j in range(4):
        W1r[:, 1 + j, :] = (0.5 * W_hh1[:, 128 * j:128 * (j + 1)].T)[:, perm] \
            * dbl
    # z2 moving chunks: [128, 5, 512]; 0..3 = W_ih2.T/2, 4 = W_hh2.T/2
    W2r = np.empty((128, 5, G2), f)
    for j in range(4):
        W2r[:, j, :] = 0.5 * W_ih2[:, 128 * j:128 * (j + 1)].T * dbl2
    W2r[:, 4, :] = 0.5 * W_hh2.T * dbl2
    WembT = np.empty((128, 4, G1), f)
    wt = (W_ih1[:, :H].T)[:, perm] * dbl          # [512, 2048]
    for kc in range(4):
        WembT[:, kc, :] = wt[128 * kc:128 * (kc + 1), :]
    WoutT = np.zeros((4, 128, 2, 2048), f)
    w0 = 0.5 * W_out[:, :KS].T
    w1 = W_out[:, KS:].T
    for g in range(4):
        gw = min(2048, V - 2048 * g)
        WoutT[g, :, 0, 0:gw] = w0[:, 2048 * g:2048 * g + gw]
        WoutT[g, :, 1, 0:gw] = w1[:, 2048 * g:2048 * g + gw]

    b1row = ((np.asarray(b_ih1, f) + np.asarray(b_hh1, f))[perm] * dbl) \
        .reshape(1, G1)
    b2row = ((np.asarray(b_ih2, f) + np.asarray(b_hh2, f)) * dbl2) \
        .reshape(1, G2)
    boutrow = np.zeros((4, 1, 2048), f)
    bo = np.asarray(b_out, f)
    for g in range(4):
        gw = min(2048, V - 2048 * g)
        boutrow[g, 0, 0:gw] = bo[2048 * g:2048 * g + gw]

    ident = np.eye(128, dtype=f)
    i8r = np.eye(8, dtype=f)
    onesr = np.ones((1, 128), f)
    onescol = np.ones((1, 128), f)
    zrow = np.zeros((1, G2), f)
    dmask = np.zeros((128, B * B), f)
    for n in range(B):
        dmask[:, B * n + n] = 1.0
    dmask8 = np.eye(8, dtype=f)
    ident48 = np.zeros((128, 8), f)
    for g in range(4):
        ident48[32 * g:32 * g + 8, :] = np.eye(8, dtype=f)

    shared = dict(emb=emb, W1r=W1r, W2r=W2r, WembT=WembT, WoutT=WoutT,
                  b1row=b1row, b2row=b2row, boutrow=boutrow,
                  ident=ident, i8r=i8r, onesr=onesr, onesb=onesr.copy(),
                  onescol=onescol, zrow=zrow, dmask=dmask, dmask8=dmask8,
                  ident48=ident48)

    per_core = []
    for c in range(NCORES):
        sl = slice(B * c, B * (c + 1))
        k_c = np.array(key[:, sl, :], f)         # (500, 8, 128)
        v_c = np.array(values[:, sl, :], f)
        lens_c = np.asarray(lens[sl])
        text_c = np.asarray(text[sl, :])
        # zero keys/vals at padded positions (t >= len)
        tmask = (np.arange(T)[:, None] < lens_c[None, :])   # (500, 8)
        k_c *= tmask[:, :, None]
        v_c *= tmask[:, :, None]
        keysT = np.ascontiguousarray(0.5 * k_c.transpose(2, 1, 0))
        # vals bf16 [128(t-in-chunk), 8, 4, 128(vs)], padded to TP=512
        v_pad = np.zeros((TP, B, VS), f)
        v_pad[:T] = v_c
        vals = np.ascontiguousarray(
            v_pad.reshape(TCH, 128, B, VS).transpose(1, 2, 0, 3))
        npad = (T - lens_c).astype(f).reshape(B, 1)
        # gather indices, t-major: idx[t*8+n] = text[n, t]; SBUF [128, mch]
        tidx = text_c.T[:nsteps].reshape(nt).astype(np.int32)
        tidx_pad = np.zeros(128 * mch, np.int32)
        tidx_pad[:nt] = tidx
        tidx_sb = np.ascontiguousarray(tidx_pad.reshape(mch, 128).T)
        ctx0T = np.ascontiguousarray(v_c[0].T)
        d = dict(shared)
        d.update(keysT=keysT, vals=vals, npad=npad, tidx=tidx_sb, ctx0T=ctx0T)
        per_core.append(d)
    return per_core


def build(ctx: ExitStack, tc, out_ap, ins, nsteps=L):
    from concourse import mybir

    nc = tc.nc
    f32 = mybir.dt.float32
    f32r = mybir.dt.float32r
    bf16 = mybir.dt.bfloat16
    AF = mybir.ActivationFunctionType
    OP = mybir.AluOpType
    nt, mch = _nt_mch(nsteps)

    mm = nc.tensor.matmul

    consts = ctx.enter_context(tc.tile_pool(name="consts", bufs=1))
    hists = ctx.enter_context(tc.tile_pool(name="hists", bufs=1))
    dram = ctx.enter_context(tc.tile_pool(name="dram", bufs=1, space="DRAM"))

    def load_const(name, dtype=f32):
        a = ins[name]
        t = consts.tile(list(a.shape), dtype, tag=name)
        nc.sync.dma_start(t[:], a[:])
        return t

    W1r = load_const("W1r", bf16)        # [128, 5, 2048]
    W2r = load_const("W2r", bf16)        # [128, 5, 512]
    keysT = load_const("keysT", bf16)    # [128, 8, 500]
    vals = load_const("vals", bf16)      # [128, 8, 4, 128] bf16
    b2row = load_const("b2row", bf16)    # [1, 512]
    ident = load_const("ident")          # [128, 128] fp32 transposes
    i8r = load_const("i8r", bf16)        # [8, 8] identity (injects)
    onesr = load_const("onesr", f32r)    # [1, 128] (phase A bias inject)
    onesb = load_const("onesb", bf16)    # [1, 128] (z2 / phase C bias)
    onescol = load_const("onescol", bf16)  # [1, 128] ones (zero-mms)
    zrow = load_const("zrow", bf16)      # [1, 512] zeros
    dmask = load_const("dmask", bf16)    # [128, 64]
    dmask8 = load_const("dmask8")        # [8, 8] fp32
    ident48 = load_const("ident48")      # [128, 8] stacked I8 blocks
    npad = load_const("npad")            # [8, 1]
    ctx0T = load_const("ctx0T", bf16)    # [128, 8]
    tidx = load_const("tidx", mybir.dt.int32)   # [128, mch]

    # histories: slot s holds the state after step s-1 (slot 0 = initial)
    zf = hists.tile([128, TP], f32)
    nc.gpsimd.memset(zf[:], 0.0)
    H2h = hists.tile([128, B * (nsteps + 1)], bf16)
    CXh = hists.tile([128, B * (nsteps + 1)], bf16)
    H1t = hists.tile([128, 2, 4, B], bf16)      # ping-pong h1T (2h scale)
    C1 = hists.tile([104, 128], f32)            # 2*c1, col-group layout
    C2 = hists.tile([B, KS], f32)               # 2*c2
    ATT = hists.tile([B, TP], bf16)             # exp(en); cols 500.. stay 0
    nc.vector.tensor_copy(H2h[:, 0:B], zf[:, 0:B])
    nc.vector.tensor_copy(CXh[:, 0:B], ctx0T[:])
    nc.vector.tensor_copy(H1t[:, 0].rearrange("p a b -> p (a b)"),
                          zf[:, 0:4 * B])
    nc.gpsimd.memset(C1[:], 0.0)
    nc.gpsimd.memset(C2[:], 0.0)
    nc.vector.tensor_copy(ATT[:], zf[0:B, 0:TP])

    zemb_d = dram.tile([128 * mch, G1], bf16)

    # ============ phase A: z_emb = emb[text] @ WembT + b1 ================
    import concourse.bass as bass
    with tc.tile_pool(name="pha", bufs=1) as pha, \
         tc.tile_pool(name="pha_g", bufs=3) as pha_g, \
         tc.tile_pool(name="pha_t", bufs=6) as pha_t, \
         tc.tile_pool(name="pha_z", bufs=2) as pha_z, \
         tc.tile_pool(name="pha_ps", bufs=3, space="PSUM") as pha_ps, \
         tc.tile_pool(name="pha_ps2", bufs=4, space="PSUM") as pha_ps2:
        WembT = pha.tile([128, 4, G1], f32r)
        nc.sync.dma_start(WembT[:], ins["WembT"][:])
        b1row = pha.tile([1, G1], f32r)
        nc.sync.dma_start(b1row[:], ins["b1row"][:])
        emb_ap = ins["emb"]

        for m in range(mch):
            gath = pha_g.tile([128, H], f32, tag="g")
            nc.gpsimd.indirect_dma_start(
                out=gath[:], out_offset=None, in_=emb_ap[:],
                in_offset=bass.IndirectOffsetOnAxis(
                    ap=tidx[:, m:m + 1], axis=0))
            zt = pha_z.tile([128, G1], bf16, tag="z")
            embT = []
            for kc in range(4):
                tp = pha_ps.tile([128, 128], f32, tag="tp")
                nc.tensor.transpose(tp[:], gath[:, 128 * kc:128 * (kc + 1)],
                                    ident[:])
                et = pha_t.tile([128, 128], f32r, tag="et")
                nc.vector.tensor_copy(et[:], tp[:])
                embT.append(et)
            for q in range(4):
                ps = pha_ps2.tile([128, 512], f32, tag="zps")
                sl = slice(512 * q, 512 * (q + 1))
                mm(ps[:], onesr[:, 0:128], b1row[:, sl],
                   start=True, stop=False)
                for kc in range(4):
                    mm(ps[:], embT[kc][:], WembT[:, kc, sl],
                       start=False, stop=(kc == 3))
                nc.scalar.copy(zt[:, sl], ps[:])
            nc.gpsimd.dma_start(zemb_d[128 * m:128 * (m + 1), :], zt[:])

    # ============ phase B: the recurrence ================================
    with tc.tile_pool(name="zemb", bufs=3) as zemb_p, \
         tc.tile_pool(name="gates", bufs=2) as gates, \
         tc.tile_pool(name="small", bufs=2) as small, \
         tc.tile_pool(name="ps_z1", bufs=1, space="PSUM") as ps_z1, \
         tc.tile_pool(name="ps_z2", bufs=1, space="PSUM") as ps_z2, \
         tc.tile_pool(name="ps_en", bufs=1, space="PSUM") as ps_en, \
         tc.tile_pool(name="ps_at", bufs=1, space="PSUM") as ps_at, \
         tc.tile_pool(name="ps_tp", bufs=1, space="PSUM") as ps_tp, \
         tc.tile_pool(name="ps_cx", bufs=1, space="PSUM") as ps_cx:

        z2_ps = ps_z2.tile([B, G2], f32)
        en_ps = ps_en.tile([B, T], f32)
        at_ps = ps_at.tile([128, 4 * B], f32)
        tp1_ps = ps_tp.tile([128, 128], f32, tag="tp1")
        tp2_ps = ps_cx.tile([128, B], f32, tag="tp2")
        cx_ps = ps_cx.tile([128, B], f32, tag="cx")
        z1a_ps = ps_z1.tile([104, 512], f32, tag="z1a")
        z1b_ps = ps_z1.tile([104, 512], f32, tag="z1b")
        z1_ps = [z1a_ps, z1b_ps]

        def z1_preblock(t, zt):
            """zero + 4 zemb injects + 16 h1 matmuls into psum bank t%2."""
            ps = z1_ps[t % 2]
            po = t % 2
            mm(ps[0:104, :], onescol[:, 0:104], zrow[:],
               start=True, stop=False, tile_position=(0, 0),
               skip_group_check=True)
            for g in range(4):
                sl = slice(512 * g, 512 * (g + 1))
                mm(ps[32 * g:32 * g + 8, :], i8r[:], zt[:, sl],
                   start=False, stop=False, tile_position=(0, 32 * g),
                   skip_group_check=True)
                for j in range(4):
                    mm(ps[32 * g:32 * g + 8, :], H1t[:, po, j, :],
                       W1r[:, 1 + j, sl],
                       start=False, stop=False, tile_position=(0, 32 * g),
                       skip_group_check=True)

        def z1_ctxwave(t):
            ps = z1_ps[t % 2]
            cxT = CXh[:, B * t:B * (t + 1)]
            for g in range(4):
                sl = slice(512 * g, 512 * (g + 1))
                mm(ps[32 * g:32 * g + 8, :], cxT, W1r[:, 0, sl],
                   start=False, stop=True, tile_position=(0, 32 * g),
                   skip_group_check=True)

        # first zemb load + first preblock
        zt0 = zemb_p.tile([B, G1], bf16, tag="zemb")
        nc.sync.dma_start(zt0[:], zemb_d[0:B, :])
        z1_preblock(0, zt0)

        for t in range(nsteps):
            pp = (t + 1) % 2
            # -- z1 ctx chunks (critical path) -------------------------
            z1_ctxwave(t)
            ps1 = z1_ps[t % 2]
            # -- gates 1 on [104, *]: one tanh + 5 ALU ops -------------
            tscr = gates.tile([104, 512], f32, tag="tscr")
            nc.scalar.activation(tscr[:], ps1[0:104, :], AF.Tanh, scale=0.5)
            ti = tscr[:, 0:128]
            tf = tscr[:, 128:256]
            tg = tscr[:, 256:384]
            to = tscr[:, 384:512]
            fc = gates.tile([104, 128], f32, tag="fc")
            u = gates.tile([104, 128], f32, tag="u")
            tcn = gates.tile([104, 128], f32, tag="tcn")
            h1s = gates.tile([104, 128], f32, tag="h1s")
            nc.vector.scalar_tensor_tensor(fc[:], tf, 1.0, C1[:],
                                           op0=OP.add, op1=OP.mult)
            nc.vector.scalar_tensor_tensor(u[:], ti, 1.0, tg,
                                           op0=OP.add, op1=OP.mult)
            nc.vector.scalar_tensor_tensor(C1[:], fc[:], 0.5, u[:],
                                           op0=OP.mult, op1=OP.add)
            nc.scalar.activation(tcn[:], C1[:], AF.Tanh, scale=0.5)
            nc.vector.scalar_tensor_tensor(h1s[:], to, 1.0, tcn[:],
                                           op0=OP.add, op1=OP.mult)
            # -- h1T: one [104,128] transpose + strided copy -----------
            nc.tensor.transpose(tp1_ps[:, 0:104], h1s[:],
                                ident[0:104, 0:104])
            nc.vector.tensor_copy(
                H1t[:, pp],
                tp1_ps[:].rearrange("p (g w) -> p g w", g=4)[:, :, 0:B])
            # -- z2 ----------------------------------------------------
            mm(z2_ps[:], onesb[:, 0:B], b2row[:], start=True, stop=False)
            for j in range(4):
                mm(z2_ps[:], H1t[:, pp, j, :], W2r[:, j, :],
                   start=False, stop=False)
            mm(z2_ps[:], H2h[:, B * t:B * (t + 1)], W2r[:, 4, :],
               start=False, stop=True)
            # -- gates 2 (row layout, [8, *]) --------------------------
            t2 = gates.tile([B, G2], f32, tag="t2")
            nc.scalar.activation(t2[:], z2_ps[:], AF.Tanh, scale=0.5)
            fc2 = small.tile([B, KS], f32, tag="fc2")
            u2 = small.tile([B, KS], f32, tag="u2")
            tcn2 = small.tile([B, KS], f32, tag="tcn2")
            h2r = small.tile([B, KS], f32, tag="h2r")
            nc.vector.scalar_tensor_tensor(fc2[:], t2[:, KS:2 * KS], 1.0,
                                           C2[:], op0=OP.add, op1=OP.mult)
            nc.vector.scalar_tensor_tensor(u2[:], t2[:, 0:KS], 1.0,
                                           t2[:, 2 * KS:3 * KS],
                                           op0=OP.add, op1=OP.mult)
            nc.vector.scalar_tensor_tensor(C2[:], fc2[:], 0.5, u2[:],
                                           op0=OP.mult, op1=OP.add)
            nc.scalar.activation(tcn2[:], C2[:], AF.Tanh, scale=0.5)
            nc.vector.scalar_tensor_tensor(h2r[:], t2[:, 3 * KS:4 * KS], 1.0,
                                           tcn2[:], op0=OP.add, op1=OP.mult)
            # -- h2T + zh2 ---------------------------------------------
            nc.tensor.transpose(tp2_ps[:], h2r[:], ident[0:8, 0:8])
            h2T = H2h[:, B * (t + 1):B * (t + 2)]
            nc.vector.tensor_copy(h2T, tp2_ps[:])
            zh2 = small.tile([128, B, B], bf16, tag="zh2")
            nc.vector.tensor_tensor(
                zh2[:],
                h2T.rearrange("p (a n) -> p a n", a=1).to_broadcast(
                    [128, B, B]),
                dmask[:].rearrange("p (a b) -> p a b", a=B), op=OP.mult)
            # -- energy: 8 matmuls, dense [8, 500] ---------------------
            for n in range(B):
                mm(en_ps[:], zh2[:, n, :], keysT[:, n, :],
                   start=(n == 0), stop=(n == B - 1))
            # -- z1 preblock for t+1 (fills PE under the softmax) ------
            if t + 1 < nsteps:
                zts = zemb_p.tile([B, G1], bf16, tag="zemb")
                nc.sync.dma_start(zts[:], zemb_d[B * (t + 1):B * (t + 2), :])
                z1_preblock(t + 1, zts)
            # -- softmax (no mask; den corrected by npad) --------------
            den = small.tile([B, 1], f32, tag="den")
            den2 = small.tile([B, 1], f32, tag="den2")
            rden = small.tile([B, 1], f32, tag="rden")
            diag8 = small.tile([B, B], bf16, tag="diag8")
            nc.scalar.activation(ATT[:, 0:T], en_ps[:], AF.Exp,
                                 accum_out=den[:])
            nc.vector.tensor_tensor(den2[:], den[:], npad[:], op=OP.subtract)
            nc.vector.reciprocal(rden[:], den2[:])
            nc.vector.tensor_scalar_mul(diag8[:], dmask8[:], rden[:, 0:1])
            # -- attT via 4 matmuls with diag(rden) --------------------
            for c in range(TCH):
                mm(at_ps[:, B * c:B * (c + 1)],
                   ATT[:, 128 * c:128 * (c + 1)], diag8[:],
                   start=True, stop=True)
            attT = small.tile([128, TCH, B], bf16, tag="attT")
            nc.vector.tensor_copy(
                attT[:].rearrange("p a b -> p (a b)"), at_ps[:])
            # -- ctx: zero + 32 bf16 matmuls -> ctxT column-wise -------
            mm(cx_ps[:], onescol[:, 0:128], zrow[:, 0:B],
               start=True, stop=False, skip_group_check=True)
            for n in range(B):
                for c in range(TCH):
                    mm(cx_ps[:, n:n + 1], vals[:, n, c, :],
                       attT[:, c, n:n + 1],
                       start=False,
                       stop=(n == B - 1 and c == TCH - 1),
                       skip_group_check=True)
            nc.vector.tensor_copy(CXh[:, B * (t + 1):B * (t + 2)], cx_ps[:])

    # ============ debug dump ============================================
    if os.environ.get("DEC_DUMP"):
        dbg = ins["dbg_out"]
        dbt = hists.tile([128, 8 * B], f32)
        nc.gpsimd.memset(dbt[:], 0.0)
        nc.vector.tensor_copy(dbt[:, 0:B], H2h[:, B:2 * B])
        nc.vector.tensor_copy(dbt[:, B:2 * B], CXh[:, B:2 * B])
        nc.vector.tensor_copy(dbt[:, 2 * B:3 * B], H1t[:, 1]
                              .rearrange("p a b -> p (a b)")[:, 0:B])
        nc.vector.tensor_copy(dbt[0:B, 3 * B:4 * B], ATT[0:B, 0:B])

    # ============ phase C: logits ========================================
    with tc.tile_pool(name="phc_w", bufs=2) as phc_w, \
         tc.tile_pool(name="phc_o", bufs=3) as phc_o, \
         tc.tile_pool(name="phc_ps", bufs=8, space="PSUM") as phc_ps:
        vchunks = [(512 * q, min(512, V - 512 * q))
                   for q in range((V + 511) // 512)]
        groups = [vchunks[i:i + 4] for i in range(0, len(vchunks), 4)]
        for gi, grp in enumerate(groups):
            g0 = grp[0][0]
            gw = sum(w for _, w in grp)
            wgf = phc_w.tile([128, 2, 2048], f32, tag="wgf")
            nc.sync.dma_start(wgf[:], ins["WoutT"][gi])
            wg = phc_w.tile([128, 2, 2048], bf16, tag="wg")
            nc.vector.tensor_copy(wg[:].rearrange("p a b -> p (a b)"),
                                  wgf[:].rearrange("p a b -> p (a b)"))
            bgf = phc_w.tile([1, 2048], f32, tag="bgf")
            nc.sync.dma_start(bgf[:], ins["boutrow"][gi])
            bg = phc_w.tile([1, 2048], bf16, tag="bg")
            nc.vector.tensor_copy(bg[:], bgf[:])
            if os.environ.get("DEC_DUMP") and g0 == 0:
                nc.vector.tensor_copy(dbt[:, 4 * B:6 * B], wg[:, 0, 0:16])
                nc.vector.tensor_copy(dbt[:, 6 * B:8 * B], wg[:, 1, 0:16])
                nc.sync.dma_start(ins["dbg_out"][:], dbt[:])
            for m in range(mch):
                rows = min(128, nt - 128 * m)
                h2blk = H2h[:, B + 128 * m:B + 128 * m + rows]
                cxblk = CXh[:, B + 128 * m:B + 128 * m + rows]
                ot = phc_o.tile([128, 2048], f32, tag="ot")
                for qi, (q0, qw) in enumerate(grp):
                    nsl = slice(q0 - g0, q0 - g0 + qw)
                    ps = phc_ps.tile([128, 512], f32, tag="lg")
                    mm(ps[0:rows, 0:qw], onesb[:, 0:rows],
                       bg[:, nsl], start=True, stop=False)
                    mm(ps[0:rows, 0:qw], h2blk, wg[:, 0, nsl],
                       start=False, stop=False)
                    mm(ps[0:rows, 0:qw], cxblk, wg[:, 1, nsl],
                       start=False, stop=True)
                    if qi % 2 == 0:
                        nc.scalar.copy(ot[0:rows, nsl], ps[0:rows, 0:qw])
                    else:
                        nc.vector.tensor_copy(ot[0:rows, nsl],
                                              ps[0:rows, 0:qw])
                nc.sync.dma_start(out_ap[128 * m:128 * m + rows, g0:g0 + gw],
                                  ot[0:rows, 0:gw])


def _build_program(nsteps):
    import concourse.tile as tile
    from concourse import bacc, mybir

    nt, mch = _nt_mch(nsteps)
    nc = bacc.Bacc("TRN2", target_bir_lowering=False, debug=False,
                   num_devices=NCORES)
    shapes = dict(
        emb=(V, H), W1r=(128, 5, G1), W2r=(128, 5, G2), WembT=(128, 4, G1),
        WoutT=(4, 128, 2, 2048), b1row=(1, G1), b2row=(1, G2), boutrow=(4, 1, 2048),
        ident=(128, 128), ident48=(128, 8), i8r=(8, 8), onesr=(1, 128),
        onesb=(1, 128), onescol=(1, 128),
        zrow=(1, G2), dmask=(128, B * B), dmask8=(8, 8), npad=(B, 1),
        keysT=(128, B, T), vals=(128, B, TCH, VS), ctx0T=(128, B),
    )
    F32R_INS = {"WembT", "b1row", "onesr"}
    BF16_INS = {"vals", "W1r", "W2r", "b2row", "onesb",
                "onescol", "zrow", "dmask", "keysT", "ctx0T", "i8r"}
    ins = {}
    for name, shp in shapes.items():
        if name in BF16_INS:
            dt_ = mybir.dt.bfloat16
        elif name in F32R_INS:
            dt_ = mybir.dt.float32r
        else:
            dt_ = mybir.dt.float32
        ins[name] = nc.dram_tensor(name, list(shp), dt_,
                                   kind="ExternalInput").ap()
    ins["tidx"] = nc.dram_tensor("tidx", [128, mch], mybir.dt.int32,
                                 kind="ExternalInput").ap()
    out = nc.dram_tensor("out", [nt, V], mybir.dt.float32,
                         kind="ExternalOutput").ap()
    if os.environ.get("DEC_DUMP"):
        ins["dbg_out"] = nc.dram_tensor("dbg_out", [128, 8 * B],
                                        mybir.dt.float32,
                                        kind="ExternalOutput").ap()
    with ExitStack() as ctx:
        tc = ctx.enter_context(tile.TileContext(nc))
        build(ctx, tc, out, ins, nsteps=nsteps)
    nc.compile()
    return nc


def kernel(**inputs) -> np.ndarray:
    from concourse.bass_utils import run_bass_kernel_spmd

    nsteps = int(os.environ.get("DEC_NSTEPS", L))
    per_core = _prep_inputs(nsteps, **inputs)
    BF = ["vals", "W1r", "W2r", "b2row", "onesb",
          "onescol", "zrow", "dmask", "keysT", "ctx0T", "i8r"]
    for d in per_core:
        for k in BF:
            d[k] = _to_bf16(d[k])
    if nsteps not in _cache:
        _cache[nsteps] = _build_program(nsteps)
    nc = _cache[nsteps]
    res = run_bass_kernel_spmd(
        nc, per_core, core_ids=list(range(NCORES)),
        trace=bool(int(os.environ.get("DEC_TRACE", "0"))),
    )
    outs = []
    for c in range(NCORES):
        o = res.results[c]["out"]        # [nt, 8000], rows t*8+n
        outs.append(o.reshape(nsteps, B, V).transpose(1, 0, 2))
    full = np.concatenate(outs, axis=0)  # (64, nsteps, 8000)
    kernel.last_results = res
    return full


def _to_bf16(a):
    import ml_dtypes
    return np.asarray(a, dtype=ml_dtypes.bfloat16)
